# revision 10
# baseline (speedup 1.0000x reference)
"""ConvPMF forward on 8 Trainium2 NeuronCores (Bass/Tile).

v3 — streamed-conv pipeline:

Host-side sharding: the valid (batch, review) pairs are split 8 ways; for
each core the host lays out its reviews' word embeddings (bf16) as a
[128 dims, n_blocks*264 words] stream — per review block: [2 zeros][256
words][6 zeros], so the zeros provide conv SAME padding and isolate
adjacent reviews.  (On-device per-word gather via SWDGE descriptors costs
~8ns/word on the Q7 — 167us/core — so gather-by-layout happens at prep
time and the device streams at DMA line rate instead.)

Per core, per 8-review chunk (2112 words, ~540KB):
  1. one contiguous HWDGE DMA chunk -> SBUF (double buffered)
  2. Conv1d as 5 PSUM-accumulated bf16 matmuls per 2-review pair
     (moving AP [2, 256] with stride-264 jump skips the zero gaps);
     4 pairs packed across PE column groups -> fm [128, 512] PSUM.
  3. max-softmax pool: max(softmax(fm)) == 1/sum_w exp(fm - max_w fm)
     -> reduce_max (DVE), Exp with accum_out (ACT), reciprocal (DVE).
Host: combine pooled vectors into item embeddings, dot with user factors.
"""
import math

import ml_dtypes
import numpy as np

import concourse.bass as bass
import concourse.mybir as mybir
import concourse.tile as tile
from concourse import bacc
from concourse.bass_utils import run_bass_kernel_spmd

f32 = mybir.dt.float32
bf16 = mybir.dt.bfloat16

N_CORES = 8
D, F, K = 128, 32, 5           # embed dim, factors (conv out channels), window
W = 256                        # words per review
BLK = 264                      # words per review block in the stream
RPC = 8                        # reviews per chunk (= one PSUM supertile)
CHUNK = RPC * BLK              # stream columns per chunk

_program_cache: dict[int, bass.Bass] = {}


def _build_program(n_sup: int) -> bass.Bass:
    nc = bacc.Bacc("TRN2", target_bir_lowering=False, debug=False)
    rev_d = nc.dram_tensor("rev", [n_sup * 128, CHUNK], bf16,
                           kind="ExternalInput")
    wt_d = nc.dram_tensor("wt", [128, K * F], bf16, kind="ExternalInput")
    pooled_d = nc.dram_tensor("pooled", [128, 2 * n_sup], f32,
                              kind="ExternalOutput")

    with tile.TileContext(nc) as tc:
        with tc.tile_pool(name="const", bufs=1) as cpool, \
             tc.tile_pool(name="gat", bufs=5) as gpool, \
             tc.tile_pool(name="wrk", bufs=2) as wpool, \
             tc.tile_pool(name="psW", bufs=1, space="PSUM") as wmpool, \
             tc.tile_pool(name="psF", bufs=4, space="PSUM") as fmpool:
            wt_sb = cpool.tile([128, K * F], bf16)
            nc.scalar.dma_start(wt_sb[:], wt_d[:])
            pooled_sb = cpool.tile([128, 2 * n_sup], f32)

            # PE warm-up stream: keeps the HAM clock-gate at full rate while
            # the first stream chunk is in flight.
            warm_ps = wmpool.tile([128, K * F], f32)
            for _ in range(16):
                nc.tensor.matmul(warm_ps[:], lhsT=wt_sb[:, 0:128],
                                 rhs=wt_sb[:], start=True, stop=True)

            for c in range(n_sup):
                rev = gpool.tile([128, CHUNK], bf16, tag="rev")
                dma_eng = nc.sync if c % 2 == 0 else nc.gpsimd
                dma_eng.dma_start(rev[:], rev_d[128 * c:128 * (c + 1), :])
                rv = rev[:].rearrange("p (r w) -> p r w", w=BLK)

                fm = fmpool.tile([128, 2 * W], f32, tag="fm")
                for k in range(K):
                    for g in range(4):
                        j = 2 * g
                        nc.tensor.matmul(
                            fm[32 * g:32 * g + 32, :],
                            lhsT=wt_sb[:, F * k:F * (k + 1)],
                            rhs=rv[:, j:j + 2, k:k + W],
                            start=(k == 0), stop=(k == K - 1),
                            tile_position=(0, 32 * g))
                for h in range(2):
                    negm = wpool.tile([128, 1], f32, tag="negm")
                    nc.vector.tensor_reduce(
                        negm[:], fm[:, W * h:W * (h + 1)],
                        axis=mybir.AxisListType.X,
                        op=mybir.AluOpType.max, negate=True)
                    e_scr = wpool.tile([128, W], bf16, tag="e")
                    ssum = wpool.tile([128, 1], f32, tag="s")
                    nc.scalar.activation(
                        e_scr[:], fm[:, W * h:W * (h + 1)],
                        mybir.ActivationFunctionType.Exp,
                        bias=negm[:], scale=1.0, accum_out=ssum[:])
                    col = 2 * c + h
                    nc.vector.reciprocal(pooled_sb[:, col:col + 1], ssum[:])
                if c == n_sup // 2:
                    lo = 2 * (n_sup // 2 + 1)
                    nc.sync.dma_start(pooled_d[:, 0:lo], pooled_sb[:, 0:lo])
            lo = 2 * (n_sup // 2 + 1)
            nc.sync.dma_start(pooled_d[:, lo:], pooled_sb[:, lo:])
    nc.compile()
    return nc


def prepare(user_indices, docs, review_counts, w_user, embed_matrix,
            conv_weight, bias):
    """Host-side sharding prep: returns (nc, in_maps, valid, n_core) or None
    when there are no valid reviews."""
    docs = np.asarray(docs)
    review_counts = np.asarray(review_counts)
    embed_bf = np.asarray(embed_matrix, dtype=np.float32).astype(
        ml_dtypes.bfloat16)
    conv_weight = np.asarray(conv_weight, dtype=np.float32)

    b_sz = docs.shape[0]
    valid = [(b, r) for b in range(b_sz) for r in range(int(review_counts[b]))]
    if not valid:
        return None

    n_core = RPC * math.ceil(len(valid) / (N_CORES * RPC))
    n_sup = n_core // RPC

    wt = np.zeros((128, K * F), dtype=ml_dtypes.bfloat16)
    for k in range(K):
        wt[:, F * k:F * (k + 1)] = conv_weight[:, :, k].T.astype(
            ml_dtypes.bfloat16)

    in_maps = []
    for c in range(N_CORES):
        revs = valid[c * n_core:(c + 1) * n_core]
        stream = np.zeros((n_sup, 128, CHUNK), dtype=ml_dtypes.bfloat16)
        if revs:
            words = np.concatenate([docs[b, r] for (b, r) in revs])
            embT = np.ascontiguousarray(embed_bf[words].T)  # [128, n*256]
            e4 = embT.reshape(128, len(revs), W)
            for sup in range(n_sup):
                r0, r1 = sup * RPC, min((sup + 1) * RPC, len(revs))
                if r0 >= r1:
                    break
                s3 = stream[sup].reshape(128, RPC, BLK)
                s3[:, :r1 - r0, 2:2 + W] = e4[:, r0:r1]
        in_maps.append({"rev": stream.reshape(n_sup * 128, CHUNK), "wt": wt})

    nc = _program_cache.get(n_sup)
    if nc is None:
        nc = _build_program(n_sup)
        _program_cache[n_sup] = nc
    return nc, in_maps, valid, n_core


def kernel(user_indices, docs, review_counts, w_user, embed_matrix, conv_weight,
           bias):
    user_indices = np.asarray(user_indices)
    docs = np.asarray(docs)
    review_counts = np.asarray(review_counts)
    w_user = np.asarray(w_user, dtype=np.float32)
    bias = np.asarray(bias, dtype=np.float32)

    b_sz = docs.shape[0]
    denom = np.maximum(review_counts, 1).astype(np.float32)
    prep = prepare(user_indices, docs, review_counts, w_user, embed_matrix,
                   conv_weight, bias)
    if prep is None:
        return np.full((b_sz,), bias[0], dtype=np.float32)
    nc, in_maps, valid, n_core = prep

    res = run_bass_kernel_spmd(nc, in_maps, list(range(N_CORES)))

    item = np.zeros((b_sz, F), dtype=np.float32)
    for i, (bb, rr) in enumerate(valid):
        c, s = i // n_core, i % n_core
        sup, r8 = s // RPC, s % RPC
        g, h = r8 // 2, r8 % 2
        pooled = res.results[c]["pooled"]
        item[bb] += pooled[32 * g:32 * g + 32, 2 * sup + h]
    item /= denom[:, None]
    out = (w_user[user_indices] * item).sum(axis=-1) + bias[0]
    return out.astype(np.float32)


# revision 14
# speedup vs baseline: 1.1495x; 1.1495x over previous
"""ConvPMF forward on 8 Trainium2 NeuronCores (Bass/Tile).

v3 — streamed-conv pipeline:

Host-side sharding: the valid (batch, review) pairs are split 8 ways; for
each core the host lays out its reviews' word embeddings (bf16) as a
[128 dims, n_blocks*264 words] stream — per review block: [2 zeros][256
words][6 zeros], so the zeros provide conv SAME padding and isolate
adjacent reviews.  (On-device per-word gather via SWDGE descriptors costs
~8ns/word on the Q7 — 167us/core — so gather-by-layout happens at prep
time and the device streams at DMA line rate instead.)

Per core, per 8-review chunk (2112 words, ~540KB):
  1. one contiguous HWDGE DMA chunk -> SBUF (double buffered)
  2. Conv1d as 5 PSUM-accumulated bf16 matmuls per 2-review pair
     (moving AP [2, 256] with stride-264 jump skips the zero gaps);
     4 pairs packed across PE column groups -> fm [128, 512] PSUM.
  3. max-softmax pool: max(softmax(fm)) == 1/sum_w exp(fm - max_w fm)
     -> reduce_max (DVE), Exp with accum_out (ACT), reciprocal (DVE).
Host: combine pooled vectors into item embeddings, dot with user factors.
"""
import math

import ml_dtypes
import numpy as np

import concourse.bass as bass
import concourse.mybir as mybir
import concourse.tile as tile
from concourse import bacc
from concourse.bass_utils import run_bass_kernel_spmd

f32 = mybir.dt.float32
bf16 = mybir.dt.bfloat16

N_CORES = 8
D, F, K = 128, 32, 5           # embed dim, factors (conv out channels), window
W = 256                        # words per review
BLK = 264                      # words per review block in the stream
RPC = 8                        # reviews per chunk (= one PSUM supertile)
CHUNK = RPC * BLK              # stream columns per chunk

_program_cache: dict[int, bass.Bass] = {}


def _build_program(n_sup: int) -> bass.Bass:
    nc = bacc.Bacc("TRN2", target_bir_lowering=False, debug=False)
    rev_d = nc.dram_tensor("rev", [n_sup * 128, CHUNK], bf16,
                           kind="ExternalInput")
    wt_d = nc.dram_tensor("wt", [128, K * F], bf16, kind="ExternalInput")
    maxm_d = nc.dram_tensor("maxm", [128, 2 * n_sup], f32,
                            kind="ExternalOutput")
    ssum_d = nc.dram_tensor("ssum", [128, 2 * n_sup], f32,
                            kind="ExternalOutput")

    with tile.TileContext(nc) as tc:
        with tc.tile_pool(name="const", bufs=1) as cpool, \
             tc.tile_pool(name="gat", bufs=5) as gpool, \
             tc.tile_pool(name="wrk", bufs=2) as wpool, \
             tc.tile_pool(name="psW", bufs=1, space="PSUM") as wmpool, \
             tc.tile_pool(name="psF", bufs=4, space="PSUM") as fmpool:
            wt_sb = cpool.tile([128, K * F], bf16)
            nc.scalar.dma_start(wt_sb[:], wt_d[:])
            maxm_sb = cpool.tile([128, 2 * n_sup], f32)
            ssum_sb = cpool.tile([128, 2 * n_sup], f32)

            # PE warm-up stream: keeps the HAM clock-gate at full rate while
            # the first stream chunk is in flight.
            warm_ps = wmpool.tile([128, K * F], f32)
            for _ in range(6):
                nc.tensor.matmul(warm_ps[:], lhsT=wt_sb[:, 0:128],
                                 rhs=wt_sb[:], start=True, stop=True)

            for c in range(n_sup):
                rev = gpool.tile([128, CHUNK], bf16, tag="rev")
                dma_eng = nc.sync if c % 2 == 0 else nc.gpsimd
                dma_eng.dma_start(rev[:], rev_d[128 * c:128 * (c + 1), :])
                rv = rev[:].rearrange("p (r w) -> p r w", w=BLK)

                fm = fmpool.tile([128, 2 * W], f32, tag="fm")
                for k in range(K):
                    for g in range(4):
                        j = 2 * g
                        nc.tensor.matmul(
                            fm[32 * g:32 * g + 32, :],
                            lhsT=wt_sb[:, F * k:F * (k + 1)],
                            rhs=rv[:, j:j + 2, k:k + W],
                            start=(k == 0), stop=(k == K - 1),
                            tile_position=(0, 32 * g))
                for h in range(2):
                    col = 2 * c + h
                    nc.vector.tensor_reduce(
                        maxm_sb[:, col:col + 1], fm[:, W * h:W * (h + 1)],
                        axis=mybir.AxisListType.X,
                        op=mybir.AluOpType.max)
                    e_scr = wpool.tile([128, W], bf16, tag="e")
                    nc.scalar.activation(
                        e_scr[:], fm[:, W * h:W * (h + 1)],
                        mybir.ActivationFunctionType.Exp,
                        accum_out=ssum_sb[:, col:col + 1])
            nc.sync.dma_start(maxm_d[:], maxm_sb[:])
            nc.sync.dma_start(ssum_d[:], ssum_sb[:])
    nc.compile()
    return nc


def prepare(user_indices, docs, review_counts, w_user, embed_matrix,
            conv_weight, bias):
    """Host-side sharding prep: returns (nc, in_maps, valid, n_core) or None
    when there are no valid reviews."""
    docs = np.asarray(docs)
    review_counts = np.asarray(review_counts)
    embed_bf = np.asarray(embed_matrix, dtype=np.float32).astype(
        ml_dtypes.bfloat16)
    conv_weight = np.asarray(conv_weight, dtype=np.float32)

    b_sz = docs.shape[0]
    valid = [(b, r) for b in range(b_sz) for r in range(int(review_counts[b]))]
    if not valid:
        return None

    n_core = RPC * math.ceil(len(valid) / (N_CORES * RPC))
    n_sup = n_core // RPC

    wt = np.zeros((128, K * F), dtype=ml_dtypes.bfloat16)
    for k in range(K):
        wt[:, F * k:F * (k + 1)] = conv_weight[:, :, k].T.astype(
            ml_dtypes.bfloat16)

    in_maps = []
    for c in range(N_CORES):
        revs = valid[c * n_core:(c + 1) * n_core]
        stream = np.zeros((n_sup, 128, CHUNK), dtype=ml_dtypes.bfloat16)
        if revs:
            words = np.concatenate([docs[b, r] for (b, r) in revs])
            embT = np.ascontiguousarray(embed_bf[words].T)  # [128, n*256]
            e4 = embT.reshape(128, len(revs), W)
            for sup in range(n_sup):
                r0, r1 = sup * RPC, min((sup + 1) * RPC, len(revs))
                if r0 >= r1:
                    break
                s3 = stream[sup].reshape(128, RPC, BLK)
                s3[:, :r1 - r0, 2:2 + W] = e4[:, r0:r1]
        in_maps.append({"rev": stream.reshape(n_sup * 128, CHUNK), "wt": wt})

    nc = _program_cache.get(n_sup)
    if nc is None:
        nc = _build_program(n_sup)
        _program_cache[n_sup] = nc
    return nc, in_maps, valid, n_core


def kernel(user_indices, docs, review_counts, w_user, embed_matrix, conv_weight,
           bias):
    user_indices = np.asarray(user_indices)
    docs = np.asarray(docs)
    review_counts = np.asarray(review_counts)
    w_user = np.asarray(w_user, dtype=np.float32)
    bias = np.asarray(bias, dtype=np.float32)

    b_sz = docs.shape[0]
    denom = np.maximum(review_counts, 1).astype(np.float32)
    prep = prepare(user_indices, docs, review_counts, w_user, embed_matrix,
                   conv_weight, bias)
    if prep is None:
        return np.full((b_sz,), bias[0], dtype=np.float32)
    nc, in_maps, valid, n_core = prep

    res = run_bass_kernel_spmd(nc, in_maps, list(range(N_CORES)))

    item = np.zeros((b_sz, F), dtype=np.float32)
    pooled_by_core = [
        (np.exp(np.asarray(res.results[c]["maxm"], dtype=np.float64))
         / np.asarray(res.results[c]["ssum"], dtype=np.float64)
         ).astype(np.float32)
        for c in range(N_CORES)]
    for i, (bb, rr) in enumerate(valid):
        c, s = i // n_core, i % n_core
        sup, r8 = s // RPC, s % RPC
        g, h = r8 // 2, r8 % 2
        item[bb] += pooled_by_core[c][32 * g:32 * g + 32, 2 * sup + h]
    item /= denom[:, None]
    out = (w_user[user_indices] * item).sum(axis=-1) + bias[0]
    return out.astype(np.float32)
